# revision 4
# baseline (speedup 1.0000x reference)
"""Trainium2 Bass kernel for nn_Cell2Tissue (scatter_memory).

Reference computation:
  avg = AvgPool4x4(Conv3x3_SAME(cell) + bias)          # (128, 64, 64)
  for each tissue sample j: ROI_j += avg               # 64x64 ROI from loc
  output = stack of B copies of the mutated tissue     # (4, 4, 128, 256, 256)

Sharding over 8 cores: core c = (sample j = c % 4, channel half h = c // 4).
Each core streams its 16MB tissue half to the output and adds its half of
avg into the dynamic 64x64 ROI. The x4 output stack is a zero-copy host
broadcast at unshard time.

Key optimizations vs the naive staging (213us -> ~126us measured):
  - bulk tissue->out copy is DRAM->DRAM: each byte occupies a DMA engine
    once instead of twice (via-SBUF 2-leg copy measured ~150 GB/s payload,
    single-queue D2D 264 GB/s, concurrent dual-queue D2D ~326 GB/s).
  - cell planes travel as fp8 (half the bytes) and are enqueued FIRST on
    both HWDGE queues; the conv chases the plane arrivals and finishes
    inside the copy window instead of serializing a ~55us tail.
  - conv runs fp8 DoubleRow matmuls: taps processed in pairs via a
    custom-stride ktile dim on the moving operand (2 taps per PE pass,
    144 passes total; the 4 corner-plane single taps pair ACROSS planes
    via a plane-pitch stride). Weights pre-scaled by a power of two into
    fp8 range; the DVE epilogue rescales while adding bias. End-to-end
    rel err ~1.6e-3 (gate 2e-2).
  - the ROI scatter is reformulated as a full-width 64-row *band*:
    tissue rows [r, r+64) are loaded early (64KB contiguous per channel),
    avg lands into the band at a dynamic column offset on the DVE
    (register-offset APs), and the band is written back whole after the
    copy. This replaces 8192 x 256B scattered ROI packets (~480
    engine-us of packet-rate-bound DMA) with ~8MB of large-packet
    traffic.
  - conv is BANK-MAJOR over 8 PSUM banks so each bank's DVE epilogue
    pipelines under the remaining matmuls.

Hardware behaviors this layout works around (measured on the axon trn2):
  - per-packet round-robin between queues lets 64KB packets starve 4KB
    plane packets ~15:1; plane loads therefore go first on both queues.
  - DGE completion semaphores tick per packet, so tile-level DMA->DMA
    gates open early; only the all-engine barrier orders reliably.
  - dynamic (register-offset) DMA streams run ~200-400 GB/s and
    serialize against each other; the band write stays one-per-queue.
"""

import os
import numpy as np

B, C, H, W = 4, 128, 256, 256
CH = C // 2          # channels per core (half)
L = 32               # half ROI width
ROI = 2 * L          # 64
NCORES = 8
PRR = 65             # polyphase plane rows (max y+pb = 64)
PRC = 66             # polyphase plane cols
PHASES = 16

_CACHE = {}


def _get_modules():
    if "mods" in _CACHE:
        return _CACHE["mods"]
    if os.environ.get("JAX_PLATFORMS") in ("cpu",):
        del os.environ["JAX_PLATFORMS"]
    import concourse.bass as bass
    import concourse.mybir as mybir
    import concourse.tile as tile
    from concourse.bass_utils import run_bass_kernel_spmd

    _CACHE["mods"] = (bass, mybir, tile, run_bass_kernel_spmd)
    return _CACHE["mods"]


def _split_multiwaits(nc, mybir, max_waits=1):
    """The walrus build here rejects >1 sem-wait on some instructions (the
    Tile tail InstDrain). Hoist extra waits onto single-wait nops placed
    immediately before, on the same engine (same-engine program order
    preserves semantics)."""
    for fn in nc.m.functions:
        for bb in fn.blocks:
            insts = bb.instructions
            i = 0
            while i < len(insts):
                inst = insts[i]
                si = inst.sync_info
                if si is not None and si.on_wait and len(si.on_wait) > max_waits:
                    waits = list(si.on_wait)
                    keep = waits[-max_waits:]
                    for k, w in enumerate(waits[:-max_waits]):
                        nop = mybir.InstNoOp(
                            name=f"{inst.name}_hoistwait_{k}",
                            sync_info=mybir.SyncInfo(on_wait=[w], on_update=[]),
                            bass_nofuse=True,
                            engine=inst.engine,
                        )
                        insts.insert(i, nop)
                        i += 1
                    si.on_wait = keep
                i += 1


# planes 0-7 stream on sync, 8-15 on scalar; consume in arrival order
_PLANE_ORDER = [0, 8, 1, 9, 2, 10, 3, 11, 4, 12, 5, 13, 6, 14, 7, 15]
_PLANE_PITCH = PRR * PRC


def _conv_groups():
    """Matmul group list in plane-arrival order. Each group:
    (ph, pb0, qb0, delta_elems, taps); taps is 1 or 2 (p, q) filter taps;
    for pairs, tap[i]'s moving operand sits at base + i*delta. The four
    corner planes (one tap each) are paired ACROSS adjacent planes via a
    plane-pitch ktile stride, so every PE pass is a DoubleRow pair."""
    rank = {ph: i for i, ph in enumerate(_PLANE_ORDER)}
    groups = []  # (ready_rank, ph, pb0, qb0, delta, taps)
    singles = []
    for pp in range(4):
        for qq in range(4):
            ph = pp * 4 + qq
            ps = [p for p in (pp, pp + 4) if p < 6]
            qs = [q for q in (qq, qq + 4) if q < 6]
            taps = [(p, q) for p in ps for q in qs]
            if len(taps) == 4:
                for q in qs:
                    groups.append((rank[ph], ph, 0, q // 4, PRC,
                                   [(pp, q), (pp + 4, q)]))
            elif len(taps) == 2:
                if len(ps) == 2:   # row pair
                    groups.append((rank[ph], ph, 0, 0, PRC,
                                   [(ps[0], qs[0]), (ps[1], qs[0])]))
                else:              # column pair
                    groups.append((rank[ph], ph, 0, 0, 1,
                                   [(ps[0], qs[0]), (ps[0], qs[1])]))
            else:
                singles.append((ph, taps[0]))
    # corner singles: (10,(2,2))+(11,(2,3)) and (14,(3,2))+(15,(3,3));
    # adjacent planes, one plane-pitch apart, same (pb,qb)=(0,0)
    for (ph_a, tap_a), (ph_b, tap_b) in zip(singles[0::2], singles[1::2]):
        assert ph_b == ph_a + 1
        groups.append((max(rank[ph_a], rank[ph_b]), ph_a, 0, 0,
                       _PLANE_PITCH, [tap_a, tap_b]))
    groups.sort(key=lambda g: g[0])
    groups = [g[1:] for g in groups]
    assert sum(len(g[4]) for g in groups) == 36
    assert all(len(g[4]) == 2 for g in groups)
    return groups


def _build_program():
    """One SPMD program: per-core inputs
      tissue (64,256,256) f32, cell (128,16,65,66) fp8 polyphase
      (replicated), w8 (128, 36*64) fp8 in _conv_groups tap order,
      biass (64,1) f32 = bias * weight_scale, roff (1,2) i32 = [r0, c0]
    output: out (64,256,256) f32 = tissue with avg added in the ROI."""
    if "nc" in _CACHE:
        return _CACHE["nc"]
    bass, mybir, tile, _ = _get_modules()
    f32 = mybir.dt.float32
    fp8 = mybir.dt.float8e4
    i32 = mybir.dt.int32

    nc = bass.Bass("TRN2", target_bir_lowering=False, debug=False,
                   num_devices=NCORES)
    tissue_d = nc.dram_tensor("tissue", (CH, H, W), f32, kind="ExternalInput").ap()
    cell_d = nc.dram_tensor("cell", (C, PHASES, PRR, PRC), fp8,
                            kind="ExternalInput").ap()
    w8_d = nc.dram_tensor("w8", (C, 36 * CH), fp8, kind="ExternalInput").ap()
    bias_d = nc.dram_tensor("bias", (CH, 1), f32, kind="ExternalInput").ap()
    roff_d = nc.dram_tensor("roff", (1, 2), i32, kind="ExternalInput").ap()
    out_d = nc.dram_tensor("out", (CH, H, W), f32, kind="ExternalOutput").ap()

    NBLK = 8             # conv row blocks: 8 output rows, one PSUM bank each
    ORB = ROI // NBLK    # 8 output rows per block
    groups = _conv_groups()
    inv_s = float(_CACHE["inv_scale"])

    with tile.TileContext(nc) as tc:
        with (
            tc.tile_pool(name="const", bufs=1) as constp,
            tc.tile_pool(name="cellp", bufs=1) as cellp,
            tc.tile_pool(name="bandp", bufs=1) as bandp,
            tc.tile_pool(name="tmp", bufs=2) as tmpp,
            tc.tile_pool(name="psum", bufs=1, space="PSUM") as psump,
        ):
            # --- small consts first on the scalar (Act HWDGE) queue ---
            roff_sb = constp.tile([1, 2], i32)
            nc.scalar.dma_start(roff_sb[:], roff_d[:])
            w_sb = constp.tile([C, 36 * CH], fp8)
            nc.scalar.dma_start(w_sb[:], w8_d[:])
            bias_sb = constp.tile([CH, 1], f32)
            nc.scalar.dma_start(bias_sb[:], bias_d[:])

            # --- cell planes: first bulk traffic on both HWDGE queues ---
            cell_t = cellp.tile([C, PHASES * PRR * PRC], fp8)
            c4 = cell_t.rearrange("c (ph r w) -> c ph r w", r=PRR, w=PRC)
            for ph in range(8):
                nc.sync.dma_start(c4[:, ph], cell_d[:, ph])
            for ph in range(8, PHASES):
                nc.scalar.dma_start(c4[:, ph], cell_d[:, ph])

            # offsets are in-bounds by construction; the runtime assert's
            # ISA op miscompiles on this walrus build
            dyn_engines = (mybir.EngineType.SP, mybir.EngineType.Activation,
                           mybir.EngineType.DVE)
            r_v = nc.values_load(roff_sb[0:1, 0:1], engines=dyn_engines,
                                 min_val=0, max_val=H - ROI,
                                 skip_runtime_bounds_check=True)
            c_v = nc.values_load(roff_sb[0:1, 1:2], engines=dyn_engines,
                                 min_val=0, max_val=W - ROI,
                                 skip_runtime_bounds_check=True)

            # --- every big-packet stream is gated on the LAST plane: 64KB
            # packets starve the 4KB plane packets ~15:1 in per-packet RR,
            # so nothing big may enter a queue until the planes land.
            # A 1-byte read of plane 15's tile makes Tile emit the wait. ---
            sy_gate = constp.tile([1, 1], fp8)
            nc.sync.dma_start(sy_gate[:], c4[0:1, 15, 0:1, 0:1])

            # --- band: tissue rows [r, r+64) full width, 64KB contiguous
            # per channel ---
            band = bandp.tile([CH, ROI * W], f32)
            b3 = band.rearrange("c (r w) -> c r w", w=W)
            nc.sync.dma_start(band[:], tissue_d[:, bass.ds(r_v, ROI), :])

            # --- bulk copy tissue -> out: DRAM->DRAM, split across both
            # HWDGE queues (concurrent dual-queue D2D measured ~326 GB/s;
            # gpsimd sw-DGE ignores the gate and starves the planes, so it
            # gets none) ---
            t2 = tissue_d.rearrange("c h w -> (c h) w")
            o2 = out_d.rearrange("c h w -> (c h) w")
            R_SY = 7680                        # sync 7.5MB (+4MB band), scalar 8.5MB
            nc.sync.dma_start(o2[0:R_SY].flatten(), t2[0:R_SY].flatten())
            nc.scalar.dma_start(o2[R_SY:].flatten(), t2[R_SY:].flatten())



            # --- conv: fp8 DoubleRow tap pairs, BANK-MAJOR so each PSUM
            # bank completes ~7.5us after the previous one and its DVE
            # epilogue pipelines underneath the remaining matmuls ---
            pss = [psump.tile([CH, ORB * ROI], f32, name=f"bank{b}")
                   for b in range(NBLK)]
            avg_f = tmpp.tile([CH, ROI * ROI], f32)
            ng = len(groups)
            for b in range(NBLK):
                woff = 0
                for gi, (ph, pb0, qb0, delta, taps) in enumerate(groups):
                    lhsT = w_sb[:, woff:woff + 2 * CH].rearrange(
                        "c (k m) -> c k m", k=2)
                    rhs = c4[:, ph, b * ORB + pb0:b * ORB + pb0 + ORB,
                             qb0:qb0 + ROI].unsqueeze(1)
                    rhs.ap[1] = [delta, 2]
                    nc.tensor.matmul(
                        pss[b][:], lhsT, rhs,
                        start=(gi == 0), stop=(gi == ng - 1),
                        perf_mode=mybir.MatmulPerfMode.DoubleRow,
                    )
                    woff += 2 * CH
                # epilogue part 1 (static, pipelined under the matmuls):
                # avg_f[b] = psum[b]/s + bias. Dynamic-AP DVE ops carry
                # ~2.4us setup each, so the dynamic landing happens ONCE
                # after the loop instead of once per bank.
                nc.vector.tensor_scalar(
                    avg_f[:, b * ORB * ROI:(b + 1) * ORB * ROI],
                    pss[b][:], bias_sb[:], inv_s,
                    mybir.AluOpType.add, mybir.AluOpType.mult,
                )

            # epilogue part 2: one dynamic-offset landing of the whole
            # 64x64 result into the band
            a3 = avg_f.rearrange("c (r w) -> c r w", w=ROI)
            dstb = b3[:, :, bass.ds(c_v, ROI)]
            nc.vector.tensor_add(dstb, dstb, a3[:])

            # --- write the band after all bulk copy writes landed ---
            tc.strict_bb_all_engine_barrier()
            band_dst = out_d[:, bass.ds(r_v, ROI), :]
            for eng, ca, cb in [(nc.sync, 0, 32), (nc.scalar, 32, 64)]:
                eng.dma_start(band_dst[ca:cb], b3[ca:cb])

    _split_multiwaits(nc, mybir)
    # gpsimd issues no DMA in this kernel; its InstDrain (an expensive
    # dge_drain) is vacuous — swap for NoOps, keeping the sem updates
    for fn in nc.m.functions:
        for bb in fn.blocks:
            insts = bb.instructions
            for i, inst in enumerate(insts):
                if (isinstance(inst, mybir.InstDrain)
                        and inst.engine == mybir.EngineType.Pool):
                    insts[i] = mybir.InstNoOp(
                        name=f"{inst.name}_nodrain",
                        sync_info=inst.sync_info,
                        bass_nofuse=True,
                        engine=inst.engine,
                    )
    _CACHE["nc"] = nc
    return nc


def _prep_inputs(tissue_features, cell_features, loc, conv_w, conv_b):
    import ml_dtypes

    fp8_np = ml_dtypes.float8_e4m3fn
    # fold AvgPool4x4 into the conv kernel: 6x6 taps
    w6 = np.zeros((C, C, 6, 6), np.float32)
    for dr in range(4):
        for dc in range(4):
            w6[:, :, dr:dr + 3, dc:dc + 3] += conv_w
    w6 *= 1.0 / 16.0

    # scale weights into fp8 range by a power of two
    wmax = float(np.abs(w6).max())
    scale = 2.0 ** int(np.floor(np.log2(224.0 / max(wmax, 1e-30))))
    _CACHE["inv_scale"] = 1.0 / scale
    w6s = w6 * scale

    # polyphase split of the zero-padded cell map:
    # plane (pp,qq)[y,x] = padded[4y+pp, 4x+qq], padded = 1px zero border
    padc = np.zeros((C, 4 * PRC, 4 * PRC), np.float32)
    padc[:, 1:1 + H, 1:1 + W] = cell_features[0]
    cell_poly = np.empty((C, PHASES, PRR, PRC), np.float32)
    for pp in range(4):
        for qq in range(4):
            cell_poly[:, pp * 4 + qq] = padc[:, pp:pp + 4 * PRR:4, qq::4]
    cell_poly = np.ascontiguousarray(cell_poly).astype(fp8_np)

    groups = _conv_groups()
    w8 = {}
    biass = {}
    for h in range(2):
        sl = slice(CH * h, CH * (h + 1))
        blocks = []
        for (ph, pb0, qb0, delta, taps) in groups:
            for (p, q) in taps:
                blocks.append(np.ascontiguousarray(w6s[sl, :, p, q].T))
        w8[h] = np.concatenate(blocks, axis=1).astype(fp8_np)  # [C, 36*CH]
        # pre-scaled so the epilogue computes (psum + bias*s) * (1/s)
        biass[h] = np.ascontiguousarray(
            conv_b[sl].astype(np.float32) / _CACHE["inv_scale"]
        ).reshape(CH, 1)

    r0 = loc[:, 1].astype(np.int64) * W // 1024 - L   # H-dim start (from loc x)
    c0 = loc[:, 0].astype(np.int64) * W // 1024 - L   # W-dim start (from loc y)

    in_maps = []
    for c in range(NCORES):
        j, h = c % B, c // B
        in_maps.append({
            "tissue": tissue_features[j, CH * h:CH * (h + 1)],
            "cell": cell_poly,
            "w8": w8[h],
            "bias": biass[h],
            "roff": np.array([[r0[j], c0[j]]], np.int32),
        })
    return in_maps


def run_device(tissue_features, cell_features, loc, conv_w, conv_b, **spmd_kwargs):
    """Build+run the SPMD kernel; returns (final (4,128,256,256), raw results)."""
    *_, run_bass_kernel_spmd = _get_modules()
    in_maps = _prep_inputs(tissue_features, cell_features, loc, conv_w, conv_b)
    # inv_scale is baked into the DVE epilogue as an immediate: rebuild if
    # a new weight tensor lands on a different power-of-two scale
    if _CACHE.get("built_scale") not in (None, _CACHE["inv_scale"]):
        _CACHE.pop("nc", None)
    nc = _build_program()
    _CACHE["built_scale"] = _CACHE["inv_scale"]
    res = run_bass_kernel_spmd(nc, in_maps, list(range(NCORES)), **spmd_kwargs)
    final = np.empty((B, C, H, W), np.float32)
    for c in range(NCORES):
        j, h = c % B, c // B
        final[j, CH * h:CH * (h + 1)] = res.results[c]["out"]
    return final, res


def kernel(tissue_features, cell_features, loc, conv_w, conv_b):
    final, _ = run_device(tissue_features, cell_features, loc, conv_w, conv_b)
    # reference stacks B copies of the fully-mutated tissue
    return np.broadcast_to(final[None], (B, B, C, H, W))


# revision 5
# speedup vs baseline: 1.3500x; 1.3500x over previous
"""Trainium2 Bass kernel for nn_Cell2Tissue (scatter_memory).

Reference computation:
  avg = AvgPool4x4(Conv3x3_SAME(cell) + bias)          # (128, 64, 64)
  for each tissue sample j: ROI_j += avg               # 64x64 ROI from loc
  output = stack of B copies of the mutated tissue     # (4, 4, 128, 256, 256)

Sharding over 8 cores: core c = (sample j = c % 4, channel half h = c // 4).
Each core streams its 16MB tissue half to the output and adds its half of
avg into the dynamic 64x64 ROI. The x4 output stack is a zero-copy host
broadcast at unshard time.

Key optimizations vs the naive staging (213us -> ~126us measured):
  - bulk tissue->out copy is DRAM->DRAM: each byte occupies a DMA engine
    once instead of twice (via-SBUF 2-leg copy measured ~150 GB/s payload,
    single-queue D2D 264 GB/s, concurrent dual-queue D2D ~326 GB/s).
  - cell planes travel as fp8 (half the bytes) and are enqueued FIRST on
    both HWDGE queues; the conv chases the plane arrivals and finishes
    inside the copy window instead of serializing a ~55us tail.
  - conv runs fp8 DoubleRow matmuls: taps processed in pairs via a
    custom-stride ktile dim on the moving operand (2 taps per PE pass,
    144 passes total; the 4 corner-plane single taps pair ACROSS planes
    via a plane-pitch stride). Weights pre-scaled by a power of two into
    fp8 range; the DVE epilogue rescales while adding bias. End-to-end
    rel err ~1.6e-3 (gate 2e-2).
  - the ROI scatter is reformulated as a full-width 64-row *band*:
    tissue rows [r, r+64) are loaded early (64KB contiguous per channel),
    avg lands into the band at a dynamic column offset on the DVE
    (register-offset APs), and the band is written back whole after the
    copy. This replaces 8192 x 256B scattered ROI packets (~480
    engine-us of packet-rate-bound DMA) with ~8MB of large-packet
    traffic.
  - conv is BANK-MAJOR over 8 PSUM banks so each bank's DVE epilogue
    pipelines under the remaining matmuls.

Hardware behaviors this layout works around (measured on the axon trn2):
  - per-packet round-robin between queues lets 64KB packets starve 4KB
    plane packets ~15:1; plane loads therefore go first on both queues.
  - DGE completion semaphores tick per packet, so tile-level DMA->DMA
    gates open early; only the all-engine barrier orders reliably.
  - dynamic (register-offset) DMA streams run ~200-400 GB/s and
    serialize against each other; the band write stays one-per-queue.
"""

import os
import numpy as np

B, C, H, W = 4, 128, 256, 256
CH = C // 2          # channels per core (half)
L = 32               # half ROI width
ROI = 2 * L          # 64
NCORES = 8
PRR = 65             # polyphase plane rows (max y+pb = 64)
PRC = 66             # polyphase plane cols
PHASES = 16

_CACHE = {}


def _get_modules():
    if "mods" in _CACHE:
        return _CACHE["mods"]
    if os.environ.get("JAX_PLATFORMS") in ("cpu",):
        del os.environ["JAX_PLATFORMS"]
    import concourse.bass as bass
    import concourse.mybir as mybir
    import concourse.tile as tile
    from concourse.bass_utils import run_bass_kernel_spmd

    _CACHE["mods"] = (bass, mybir, tile, run_bass_kernel_spmd)
    return _CACHE["mods"]


def _split_multiwaits(nc, mybir, max_waits=1):
    """The walrus build here rejects >1 sem-wait on some instructions (the
    Tile tail InstDrain). Hoist extra waits onto single-wait nops placed
    immediately before, on the same engine (same-engine program order
    preserves semantics)."""
    for fn in nc.m.functions:
        for bb in fn.blocks:
            insts = bb.instructions
            i = 0
            while i < len(insts):
                inst = insts[i]
                si = inst.sync_info
                if si is not None and si.on_wait and len(si.on_wait) > max_waits:
                    waits = list(si.on_wait)
                    keep = waits[-max_waits:]
                    for k, w in enumerate(waits[:-max_waits]):
                        nop = mybir.InstNoOp(
                            name=f"{inst.name}_hoistwait_{k}",
                            sync_info=mybir.SyncInfo(on_wait=[w], on_update=[]),
                            bass_nofuse=True,
                            engine=inst.engine,
                        )
                        insts.insert(i, nop)
                        i += 1
                    si.on_wait = keep
                i += 1


# planes 0-7 stream on sync, 8-15 on scalar; consume in arrival order
_PLANE_ORDER = [0, 8, 1, 9, 2, 10, 3, 11, 4, 12, 5, 13, 6, 14, 7, 15]
_PLANE_PITCH = PRR * PRC


def _conv_groups():
    """Matmul group list in plane-arrival order. Each group:
    (ph, pb0, qb0, delta_elems, taps); taps is 1 or 2 (p, q) filter taps;
    for pairs, tap[i]'s moving operand sits at base + i*delta. The four
    corner planes (one tap each) are paired ACROSS adjacent planes via a
    plane-pitch ktile stride, so every PE pass is a DoubleRow pair."""
    rank = {ph: i for i, ph in enumerate(_PLANE_ORDER)}
    groups = []  # (ready_rank, ph, pb0, qb0, delta, taps)
    singles = []
    for pp in range(4):
        for qq in range(4):
            ph = pp * 4 + qq
            ps = [p for p in (pp, pp + 4) if p < 6]
            qs = [q for q in (qq, qq + 4) if q < 6]
            taps = [(p, q) for p in ps for q in qs]
            if len(taps) == 4:
                for q in qs:
                    groups.append((rank[ph], ph, 0, q // 4, PRC,
                                   [(pp, q), (pp + 4, q)]))
            elif len(taps) == 2:
                if len(ps) == 2:   # row pair
                    groups.append((rank[ph], ph, 0, 0, PRC,
                                   [(ps[0], qs[0]), (ps[1], qs[0])]))
                else:              # column pair
                    groups.append((rank[ph], ph, 0, 0, 1,
                                   [(ps[0], qs[0]), (ps[0], qs[1])]))
            else:
                singles.append((ph, taps[0]))
    # corner singles: (10,(2,2))+(11,(2,3)) and (14,(3,2))+(15,(3,3));
    # adjacent planes, one plane-pitch apart, same (pb,qb)=(0,0)
    for (ph_a, tap_a), (ph_b, tap_b) in zip(singles[0::2], singles[1::2]):
        assert ph_b == ph_a + 1
        groups.append((max(rank[ph_a], rank[ph_b]), ph_a, 0, 0,
                       _PLANE_PITCH, [tap_a, tap_b]))
    groups.sort(key=lambda g: g[0])
    groups = [g[1:] for g in groups]
    assert sum(len(g[4]) for g in groups) == 36
    assert all(len(g[4]) == 2 for g in groups)
    return groups


def _build_program():
    """One SPMD program: per-core inputs
      tissue (64,256,256) f32, cell (128,16,65,66) fp8 polyphase
      (replicated), w8 (128, 36*64) fp8 in _conv_groups tap order,
      biass (64,1) f32 = bias * weight_scale, roff (1,2) i32 = [r0, c0]
    output: out (64,256,256) f32 = tissue with avg added in the ROI."""
    if "nc" in _CACHE:
        return _CACHE["nc"]
    bass, mybir, tile, _ = _get_modules()
    f32 = mybir.dt.float32
    fp8 = mybir.dt.float8e4
    i32 = mybir.dt.int32

    nc = bass.Bass("TRN2", target_bir_lowering=False, debug=False,
                   num_devices=NCORES)
    tissue_d = nc.dram_tensor("tissue", (CH, H, W), f32, kind="ExternalInput").ap()
    cell_d = nc.dram_tensor("cell", (C, PHASES, PRR, PRC), fp8,
                            kind="ExternalInput").ap()
    w8_d = nc.dram_tensor("w8", (C, 36 * CH), fp8, kind="ExternalInput").ap()
    bias_d = nc.dram_tensor("bias", (CH, 1), f32, kind="ExternalInput").ap()
    roff_d = nc.dram_tensor("roff", (1, 2), i32, kind="ExternalInput").ap()
    out_d = nc.dram_tensor("out", (CH, H, W), f32, kind="ExternalOutput").ap()

    NBLK = 8             # conv row blocks: 8 output rows, one PSUM bank each
    ORB = ROI // NBLK    # 8 output rows per block
    groups = _conv_groups()
    inv_s = float(_CACHE["inv_scale"])

    with tile.TileContext(nc) as tc:
        with (
            tc.tile_pool(name="const", bufs=1) as constp,
            tc.tile_pool(name="cellp", bufs=1) as cellp,
            tc.tile_pool(name="bandp", bufs=1) as bandp,
            tc.tile_pool(name="tmp", bufs=2) as tmpp,
            tc.tile_pool(name="psum", bufs=1, space="PSUM") as psump,
        ):
            # --- small consts first on the scalar (Act HWDGE) queue ---
            roff_sb = constp.tile([1, 2], i32)
            nc.scalar.dma_start(roff_sb[:], roff_d[:])
            w_sb = constp.tile([C, 36 * CH], fp8)
            nc.scalar.dma_start(w_sb[:], w8_d[:])
            bias_sb = constp.tile([CH, 1], f32)
            nc.scalar.dma_start(bias_sb[:], bias_d[:])

            # --- cell planes: first bulk traffic on both HWDGE queues ---
            cell_t = cellp.tile([C, PHASES * PRR * PRC], fp8)
            c4 = cell_t.rearrange("c (ph r w) -> c ph r w", r=PRR, w=PRC)
            for ph in range(8):
                nc.sync.dma_start(c4[:, ph], cell_d[:, ph])
            for ph in range(8, PHASES):
                nc.scalar.dma_start(c4[:, ph], cell_d[:, ph])

            # offsets are in-bounds by construction; the runtime assert's
            # ISA op miscompiles on this walrus build
            dyn_engines = (mybir.EngineType.SP, mybir.EngineType.Activation,
                           mybir.EngineType.DVE)
            r_v = nc.values_load(roff_sb[0:1, 0:1], engines=dyn_engines,
                                 min_val=0, max_val=H - ROI,
                                 skip_runtime_bounds_check=True)
            c_v = nc.values_load(roff_sb[0:1, 1:2], engines=dyn_engines,
                                 min_val=0, max_val=W - ROI,
                                 skip_runtime_bounds_check=True)

            # --- every big-packet stream is gated on the LAST plane: 64KB
            # packets starve the 4KB plane packets ~15:1 in per-packet RR,
            # so nothing big may enter a queue until the planes land.
            # A 1-byte read of plane 15's tile makes Tile emit the wait. ---
            sy_gate = constp.tile([1, 1], fp8)
            nc.sync.dma_start(sy_gate[:], c4[0:1, 15, 0:1, 0:1])

            # --- band: tissue rows [r, r+64) full width, 64KB contiguous
            # per channel ---
            band = bandp.tile([CH, ROI * W], f32)
            b3 = band.rearrange("c (r w) -> c r w", w=W)
            nc.sync.dma_start(band[:], tissue_d[:, bass.ds(r_v, ROI), :])

            # --- bulk copy tissue -> out: DRAM->DRAM, split across both
            # HWDGE queues (concurrent dual-queue D2D measured ~326 GB/s;
            # gpsimd sw-DGE ignores the gate and starves the planes, so it
            # gets none) ---
            t2 = tissue_d.rearrange("c h w -> (c h) w")
            o2 = out_d.rearrange("c h w -> (c h) w")
            R_SY = 6144                        # sync 6MB (+4MB band), scalar 10MB
            nc.sync.dma_start(o2[0:R_SY].flatten(), t2[0:R_SY].flatten())
            nc.scalar.dma_start(o2[R_SY:].flatten(), t2[R_SY:].flatten())



            # --- conv: fp8 DoubleRow tap pairs, BANK-MAJOR so each PSUM
            # bank completes ~7.5us after the previous one and its DVE
            # epilogue pipelines underneath the remaining matmuls ---
            pss = [psump.tile([CH, ORB * ROI], f32, name=f"bank{b}")
                   for b in range(NBLK)]
            avg_f = tmpp.tile([CH, ROI * ROI], f32)
            ng = len(groups)
            for b in range(NBLK):
                woff = 0
                for gi, (ph, pb0, qb0, delta, taps) in enumerate(groups):
                    lhsT = w_sb[:, woff:woff + 2 * CH].rearrange(
                        "c (k m) -> c k m", k=2)
                    rhs = c4[:, ph, b * ORB + pb0:b * ORB + pb0 + ORB,
                             qb0:qb0 + ROI].unsqueeze(1)
                    rhs.ap[1] = [delta, 2]
                    nc.tensor.matmul(
                        pss[b][:], lhsT, rhs,
                        start=(gi == 0), stop=(gi == ng - 1),
                        perf_mode=mybir.MatmulPerfMode.DoubleRow,
                    )
                    woff += 2 * CH
                # epilogue part 1 (static, pipelined under the matmuls):
                # avg_f[b] = psum[b]/s + bias. Dynamic-AP DVE ops carry
                # ~2.4us setup each, so the dynamic landing happens ONCE
                # after the loop instead of once per bank.
                nc.vector.tensor_scalar(
                    avg_f[:, b * ORB * ROI:(b + 1) * ORB * ROI],
                    pss[b][:], bias_sb[:], inv_s,
                    mybir.AluOpType.add, mybir.AluOpType.mult,
                )

            # epilogue part 2: one dynamic-offset landing of the whole
            # 64x64 result into the band
            a3 = avg_f.rearrange("c (r w) -> c r w", w=ROI)
            dstb = b3[:, :, bass.ds(c_v, ROI)]
            nc.vector.tensor_add(dstb, dstb, a3[:])

            # --- write the band after all bulk copy writes landed ---
            tc.strict_bb_all_engine_barrier()
            band_dst = out_d[:, bass.ds(r_v, ROI), :]
            for eng, ca, cb in [(nc.sync, 0, 32), (nc.scalar, 32, 64)]:
                eng.dma_start(band_dst[ca:cb], b3[ca:cb])

    _split_multiwaits(nc, mybir)
    # gpsimd issues no DMA in this kernel; its InstDrain (an expensive
    # dge_drain) is vacuous — swap for NoOps, keeping the sem updates
    for fn in nc.m.functions:
        for bb in fn.blocks:
            insts = bb.instructions
            for i, inst in enumerate(insts):
                if (isinstance(inst, mybir.InstDrain)
                        and inst.engine == mybir.EngineType.Pool):
                    insts[i] = mybir.InstNoOp(
                        name=f"{inst.name}_nodrain",
                        sync_info=inst.sync_info,
                        bass_nofuse=True,
                        engine=inst.engine,
                    )
    _CACHE["nc"] = nc
    return nc


def _prep_inputs(tissue_features, cell_features, loc, conv_w, conv_b):
    import ml_dtypes

    fp8_np = ml_dtypes.float8_e4m3fn
    # fold AvgPool4x4 into the conv kernel: 6x6 taps
    w6 = np.zeros((C, C, 6, 6), np.float32)
    for dr in range(4):
        for dc in range(4):
            w6[:, :, dr:dr + 3, dc:dc + 3] += conv_w
    w6 *= 1.0 / 16.0

    # scale weights into fp8 range by a power of two
    wmax = float(np.abs(w6).max())
    scale = 2.0 ** int(np.floor(np.log2(224.0 / max(wmax, 1e-30))))
    _CACHE["inv_scale"] = 1.0 / scale
    w6s = w6 * scale

    # polyphase split of the zero-padded cell map:
    # plane (pp,qq)[y,x] = padded[4y+pp, 4x+qq], padded = 1px zero border
    padc = np.zeros((C, 4 * PRC, 4 * PRC), np.float32)
    padc[:, 1:1 + H, 1:1 + W] = cell_features[0]
    cell_poly = np.empty((C, PHASES, PRR, PRC), np.float32)
    for pp in range(4):
        for qq in range(4):
            cell_poly[:, pp * 4 + qq] = padc[:, pp:pp + 4 * PRR:4, qq::4]
    cell_poly = np.ascontiguousarray(cell_poly).astype(fp8_np)

    groups = _conv_groups()
    w8 = {}
    biass = {}
    for h in range(2):
        sl = slice(CH * h, CH * (h + 1))
        blocks = []
        for (ph, pb0, qb0, delta, taps) in groups:
            for (p, q) in taps:
                blocks.append(np.ascontiguousarray(w6s[sl, :, p, q].T))
        w8[h] = np.concatenate(blocks, axis=1).astype(fp8_np)  # [C, 36*CH]
        # pre-scaled so the epilogue computes (psum + bias*s) * (1/s)
        biass[h] = np.ascontiguousarray(
            conv_b[sl].astype(np.float32) / _CACHE["inv_scale"]
        ).reshape(CH, 1)

    r0 = loc[:, 1].astype(np.int64) * W // 1024 - L   # H-dim start (from loc x)
    c0 = loc[:, 0].astype(np.int64) * W // 1024 - L   # W-dim start (from loc y)

    in_maps = []
    for c in range(NCORES):
        j, h = c % B, c // B
        in_maps.append({
            "tissue": tissue_features[j, CH * h:CH * (h + 1)],
            "cell": cell_poly,
            "w8": w8[h],
            "bias": biass[h],
            "roff": np.array([[r0[j], c0[j]]], np.int32),
        })
    return in_maps


def run_device(tissue_features, cell_features, loc, conv_w, conv_b, **spmd_kwargs):
    """Build+run the SPMD kernel; returns (final (4,128,256,256), raw results)."""
    *_, run_bass_kernel_spmd = _get_modules()
    in_maps = _prep_inputs(tissue_features, cell_features, loc, conv_w, conv_b)
    # inv_scale is baked into the DVE epilogue as an immediate: rebuild if
    # a new weight tensor lands on a different power-of-two scale
    if _CACHE.get("built_scale") not in (None, _CACHE["inv_scale"]):
        _CACHE.pop("nc", None)
    nc = _build_program()
    _CACHE["built_scale"] = _CACHE["inv_scale"]
    res = run_bass_kernel_spmd(nc, in_maps, list(range(NCORES)), **spmd_kwargs)
    final = np.empty((B, C, H, W), np.float32)
    for c in range(NCORES):
        j, h = c % B, c // B
        final[j, CH * h:CH * (h + 1)] = res.results[c]["out"]
    return final, res


def kernel(tissue_features, cell_features, loc, conv_w, conv_b):
    final, _ = run_device(tissue_features, cell_features, loc, conv_w, conv_b)
    # reference stacks B copies of the fully-mutated tissue
    return np.broadcast_to(final[None], (B, B, C, H, W))


# revision 7
# speedup vs baseline: 1.4769x; 1.0940x over previous
"""Trainium2 Bass kernel for nn_Cell2Tissue (scatter_memory).

Reference computation:
  avg = AvgPool4x4(Conv3x3_SAME(cell) + bias)          # (128, 64, 64)
  for each tissue sample j: ROI_j += avg               # 64x64 ROI from loc
  output = stack of B copies of the mutated tissue     # (4, 4, 128, 256, 256)

Sharding over 8 cores: core c = (sample j = c % 4, channel half h = c // 4).
Each core streams its 16MB tissue half to the output and adds its half of
avg into the dynamic 64x64 ROI. The x4 output stack is a zero-copy host
broadcast at unshard time.

Key optimizations vs the naive staging (213us -> ~126us measured):
  - bulk tissue->out copy is DRAM->DRAM: each byte occupies a DMA engine
    once instead of twice (via-SBUF 2-leg copy measured ~150 GB/s payload,
    single-queue D2D 264 GB/s, concurrent dual-queue D2D ~326 GB/s).
  - cell planes travel as fp8 (half the bytes) and are enqueued FIRST on
    both HWDGE queues; the conv chases the plane arrivals and finishes
    inside the copy window instead of serializing a ~55us tail.
  - conv runs fp8 DoubleRow matmuls: taps processed in pairs via a
    custom-stride ktile dim on the moving operand (2 taps per PE pass,
    144 passes total; the 4 corner-plane single taps pair ACROSS planes
    via a plane-pitch stride). Weights pre-scaled by a power of two into
    fp8 range; the DVE epilogue rescales while adding bias. End-to-end
    rel err ~1.6e-3 (gate 2e-2).
  - the ROI scatter is reformulated as a full-width 64-row *band*:
    tissue rows [r, r+64) are loaded early (64KB contiguous per channel),
    avg lands into the band at a dynamic column offset on the DVE
    (register-offset APs), and the band is written back whole after the
    copy. This replaces 8192 x 256B scattered ROI packets (~480
    engine-us of packet-rate-bound DMA) with ~8MB of large-packet
    traffic.
  - conv is BANK-MAJOR over 8 PSUM banks so each bank's DVE epilogue
    pipelines under the remaining matmuls.

Hardware behaviors this layout works around (measured on the axon trn2):
  - per-packet round-robin between queues lets 64KB packets starve 4KB
    plane packets ~15:1; plane loads therefore go first on both queues.
  - DGE completion semaphores tick per packet, so tile-level DMA->DMA
    gates open early; only the all-engine barrier orders reliably.
  - dynamic (register-offset) DMA streams run ~200-400 GB/s and
    serialize against each other; the band write stays one-per-queue.
"""

import os
import numpy as np

B, C, H, W = 4, 128, 256, 256
CH = C // 2          # channels per core (half)
L = 32               # half ROI width
ROI = 2 * L          # 64
NCORES = 8
PRR = 65             # polyphase plane rows (max y+pb = 64)
PRC = 66             # polyphase plane cols
PHASES = 16

_CACHE = {}


def _get_modules():
    if "mods" in _CACHE:
        return _CACHE["mods"]
    if os.environ.get("JAX_PLATFORMS") in ("cpu",):
        del os.environ["JAX_PLATFORMS"]
    import concourse.bass as bass
    import concourse.mybir as mybir
    import concourse.tile as tile
    from concourse.bass_utils import run_bass_kernel_spmd

    _CACHE["mods"] = (bass, mybir, tile, run_bass_kernel_spmd)
    return _CACHE["mods"]


def _split_multiwaits(nc, mybir, max_waits=1):
    """The walrus build here rejects >1 sem-wait on some instructions (the
    Tile tail InstDrain). Hoist extra waits onto single-wait nops placed
    immediately before, on the same engine (same-engine program order
    preserves semantics)."""
    for fn in nc.m.functions:
        for bb in fn.blocks:
            insts = bb.instructions
            i = 0
            while i < len(insts):
                inst = insts[i]
                si = inst.sync_info
                if si is not None and si.on_wait and len(si.on_wait) > max_waits:
                    waits = list(si.on_wait)
                    keep = waits[-max_waits:]
                    for k, w in enumerate(waits[:-max_waits]):
                        nop = mybir.InstNoOp(
                            name=f"{inst.name}_hoistwait_{k}",
                            sync_info=mybir.SyncInfo(on_wait=[w], on_update=[]),
                            bass_nofuse=True,
                            engine=inst.engine,
                        )
                        insts.insert(i, nop)
                        i += 1
                    si.on_wait = keep
                i += 1


# planes 0-7 stream on sync, 8-15 on scalar; consume in arrival order
_PLANE_ORDER = [0, 8, 1, 9, 2, 10, 3, 11, 4, 12, 5, 13, 6, 14, 7, 15]
_PLANE_PITCH = PRR * PRC


def _conv_groups():
    """Matmul group list in plane-arrival order. Each group:
    (ph, pb0, qb0, delta_elems, taps); taps is 1 or 2 (p, q) filter taps;
    for pairs, tap[i]'s moving operand sits at base + i*delta. The four
    corner planes (one tap each) are paired ACROSS adjacent planes via a
    plane-pitch ktile stride, so every PE pass is a DoubleRow pair."""
    rank = {ph: i for i, ph in enumerate(_PLANE_ORDER)}
    groups = []  # (ready_rank, ph, pb0, qb0, delta, taps)
    singles = []
    for pp in range(4):
        for qq in range(4):
            ph = pp * 4 + qq
            ps = [p for p in (pp, pp + 4) if p < 6]
            qs = [q for q in (qq, qq + 4) if q < 6]
            taps = [(p, q) for p in ps for q in qs]
            if len(taps) == 4:
                for q in qs:
                    groups.append((rank[ph], ph, 0, q // 4, PRC,
                                   [(pp, q), (pp + 4, q)]))
            elif len(taps) == 2:
                if len(ps) == 2:   # row pair
                    groups.append((rank[ph], ph, 0, 0, PRC,
                                   [(ps[0], qs[0]), (ps[1], qs[0])]))
                else:              # column pair
                    groups.append((rank[ph], ph, 0, 0, 1,
                                   [(ps[0], qs[0]), (ps[0], qs[1])]))
            else:
                singles.append((ph, taps[0]))
    # corner singles: (10,(2,2))+(11,(2,3)) and (14,(3,2))+(15,(3,3));
    # adjacent planes, one plane-pitch apart, same (pb,qb)=(0,0)
    for (ph_a, tap_a), (ph_b, tap_b) in zip(singles[0::2], singles[1::2]):
        assert ph_b == ph_a + 1
        groups.append((max(rank[ph_a], rank[ph_b]), ph_a, 0, 0,
                       _PLANE_PITCH, [tap_a, tap_b]))
    groups.sort(key=lambda g: g[0])
    groups = [g[1:] for g in groups]
    assert sum(len(g[4]) for g in groups) == 36
    assert all(len(g[4]) == 2 for g in groups)
    return groups


def _build_program():
    """One SPMD program: per-core inputs
      tissue (64,256,256) f32, cell (128,16,65,66) fp8 polyphase
      (replicated), w8 (128, 36*64) fp8 in _conv_groups tap order,
      biass (64,1) f32 = bias * weight_scale, roff (1,2) i32 = [r0, c0]
    output: out (64,256,256) f32 = tissue with avg added in the ROI."""
    if "nc" in _CACHE:
        return _CACHE["nc"]
    bass, mybir, tile, _ = _get_modules()
    f32 = mybir.dt.float32
    fp8 = mybir.dt.float8e4
    i32 = mybir.dt.int32

    nc = bass.Bass("TRN2", target_bir_lowering=False, debug=False,
                   num_devices=NCORES)
    tissue_d = nc.dram_tensor("tissue", (CH, H, W), f32, kind="ExternalInput").ap()
    cell_d = nc.dram_tensor("cell", (C, PHASES, PRR, PRC), fp8,
                            kind="ExternalInput").ap()
    w8_d = nc.dram_tensor("w8", (C, 36 * CH), fp8, kind="ExternalInput").ap()
    bias_d = nc.dram_tensor("bias", (CH, 1), f32, kind="ExternalInput").ap()
    roff_d = nc.dram_tensor("roff", (1, 2), i32, kind="ExternalInput").ap()
    out_d = nc.dram_tensor("out", (CH, H, W), f32, kind="ExternalOutput").ap()

    NBLK = 8             # conv row blocks: 8 output rows, one PSUM bank each
    ORB = ROI // NBLK    # 8 output rows per block
    groups = _conv_groups()
    inv_s = float(_CACHE["inv_scale"])

    with tile.TileContext(nc) as tc:
        with (
            tc.tile_pool(name="const", bufs=1) as constp,
            tc.tile_pool(name="cellp", bufs=1) as cellp,
            tc.tile_pool(name="bandp", bufs=1) as bandp,
            tc.tile_pool(name="tmp", bufs=2) as tmpp,
            tc.tile_pool(name="psum", bufs=1, space="PSUM") as psump,
        ):
            # --- small consts first on the scalar (Act HWDGE) queue ---
            roff_sb = constp.tile([1, 2], i32)
            nc.scalar.dma_start(roff_sb[:], roff_d[:])
            w_sb = constp.tile([C, 36 * CH], fp8)
            nc.scalar.dma_start(w_sb[:], w8_d[:])
            bias_sb = constp.tile([CH, 1], f32)
            nc.scalar.dma_start(bias_sb[:], bias_d[:])

            # --- cell planes: first bulk traffic on both HWDGE queues ---
            cell_t = cellp.tile([C, PHASES * PRR * PRC], fp8)
            c4 = cell_t.rearrange("c (ph r w) -> c ph r w", r=PRR, w=PRC)
            for ph in range(8):
                nc.sync.dma_start(c4[:, ph], cell_d[:, ph])
            for ph in range(8, PHASES):
                nc.scalar.dma_start(c4[:, ph], cell_d[:, ph])

            # offsets are in-bounds by construction; the runtime assert's
            # ISA op miscompiles on this walrus build
            dyn_engines = (mybir.EngineType.SP, mybir.EngineType.Activation,
                           mybir.EngineType.DVE)
            r_v = nc.values_load(roff_sb[0:1, 0:1], engines=dyn_engines,
                                 min_val=0, max_val=H - ROI,
                                 skip_runtime_bounds_check=True)
            c_v = nc.values_load(roff_sb[0:1, 1:2], engines=dyn_engines,
                                 min_val=0, max_val=W - ROI,
                                 skip_runtime_bounds_check=True)

            # --- every big-packet stream is gated on the LAST plane: 64KB
            # packets starve the 4KB plane packets ~15:1 in per-packet RR,
            # so nothing big may enter a queue until the planes land.
            # A 1-byte read of plane 15's tile makes Tile emit the wait. ---
            sy_gate = constp.tile([1, 1], fp8)
            nc.sync.dma_start(sy_gate[:], c4[0:1, 15, 0:1, 0:1])

            # --- band: tissue rows [r, r+64) full width, 64KB contiguous
            # per channel ---
            band = bandp.tile([CH, ROI * W], f32)
            b3 = band.rearrange("c (r w) -> c r w", w=W)
            nc.sync.dma_start(band[:], tissue_d[:, bass.ds(r_v, ROI), :])

            # --- bulk copy tissue -> out: DRAM->DRAM, split across both
            # HWDGE queues (concurrent dual-queue D2D measured ~326 GB/s;
            # gpsimd sw-DGE ignores the gate and starves the planes, so it
            # gets none) ---
            t2 = tissue_d.rearrange("c h w -> (c h) w")
            o2 = out_d.rearrange("c h w -> (c h) w")
            R_SY = 6144                        # sync 6MB (+4MB band), scalar 10MB
            nc.sync.dma_start(o2[0:R_SY].flatten(), t2[0:R_SY].flatten())
            nc.scalar.dma_start(o2[R_SY:].flatten(), t2[R_SY:].flatten())



            # --- conv: fp8 DoubleRow tap pairs, BANK-MAJOR so each PSUM
            # bank completes ~7.5us after the previous one and its DVE
            # epilogue pipelines underneath the remaining matmuls ---
            pss = [psump.tile([CH, ORB * ROI], f32, name=f"bank{b}")
                   for b in range(NBLK)]
            avg_f = tmpp.tile([CH, ROI * ROI], f32)
            ng = len(groups)
            for b in range(NBLK):
                woff = 0
                for gi, (ph, pb0, qb0, delta, taps) in enumerate(groups):
                    lhsT = w_sb[:, woff:woff + 2 * CH].rearrange(
                        "c (k m) -> c k m", k=2)
                    rhs = c4[:, ph, b * ORB + pb0:b * ORB + pb0 + ORB,
                             qb0:qb0 + ROI].unsqueeze(1)
                    rhs.ap[1] = [delta, 2]
                    nc.tensor.matmul(
                        pss[b][:], lhsT, rhs,
                        start=(gi == 0), stop=(gi == ng - 1),
                        perf_mode=mybir.MatmulPerfMode.DoubleRow,
                    )
                    woff += 2 * CH
                # epilogue part 1 (static, pipelined under the matmuls):
                # avg_f[b] = psum[b]/s + bias. Dynamic-AP DVE ops carry
                # ~2.4us setup each, so the dynamic landing happens ONCE
                # after the loop instead of once per bank.
                nc.vector.tensor_scalar(
                    avg_f[:, b * ORB * ROI:(b + 1) * ORB * ROI],
                    pss[b][:], bias_sb[:], inv_s,
                    mybir.AluOpType.add, mybir.AluOpType.mult,
                )

            # epilogue part 2: one dynamic-offset landing of the whole
            # 64x64 result into the band
            a3 = avg_f.rearrange("c (r w) -> c r w", w=ROI)
            dstb = b3[:, :, bass.ds(c_v, ROI)]
            nc.vector.tensor_add(dstb, dstb, a3[:])

            # --- write the band after all bulk copy writes landed ---
            tc.strict_bb_all_engine_barrier()
            band_dst = out_d[:, bass.ds(r_v, ROI), :]
            for eng, ca, cb in [(nc.sync, 0, 32), (nc.scalar, 32, 64)]:
                eng.dma_start(band_dst[ca:cb], b3[ca:cb])

    _split_multiwaits(nc, mybir)
    # gpsimd issues no DMA in this kernel; its InstDrain (an expensive
    # dge_drain) is vacuous — swap for NoOps, keeping the sem updates
    for fn in nc.m.functions:
        for bb in fn.blocks:
            insts = bb.instructions
            for i, inst in enumerate(insts):
                if (isinstance(inst, mybir.InstDrain)
                        and inst.engine == mybir.EngineType.Pool):
                    insts[i] = mybir.InstNoOp(
                        name=f"{inst.name}_nodrain",
                        sync_info=inst.sync_info,
                        bass_nofuse=True,
                        engine=inst.engine,
                    )
    _CACHE["nc"] = nc
    return nc


def _prep_inputs(tissue_features, cell_features, loc, conv_w, conv_b):
    import ml_dtypes

    fp8_np = ml_dtypes.float8_e4m3fn
    # fold AvgPool4x4 into the conv kernel: 6x6 taps
    w6 = np.zeros((C, C, 6, 6), np.float32)
    for dr in range(4):
        for dc in range(4):
            w6[:, :, dr:dr + 3, dc:dc + 3] += conv_w
    w6 *= 1.0 / 16.0

    # scale weights into fp8 range by a power of two
    wmax = float(np.abs(w6).max())
    scale = 2.0 ** int(np.floor(np.log2(224.0 / max(wmax, 1e-30))))
    _CACHE["inv_scale"] = 1.0 / scale
    w6s = w6 * scale

    # polyphase split of the zero-padded cell map:
    # plane (pp,qq)[y,x] = padded[4y+pp, 4x+qq], padded = 1px zero border
    padc = np.zeros((C, 4 * PRC, 4 * PRC), np.float32)
    padc[:, 1:1 + H, 1:1 + W] = cell_features[0]
    cell_poly = np.empty((C, PHASES, PRR, PRC), np.float32)
    for pp in range(4):
        for qq in range(4):
            cell_poly[:, pp * 4 + qq] = padc[:, pp:pp + 4 * PRR:4, qq::4]
    cell_poly = np.ascontiguousarray(cell_poly).astype(fp8_np)

    groups = _conv_groups()
    w8 = {}
    biass = {}
    for h in range(2):
        sl = slice(CH * h, CH * (h + 1))
        blocks = []
        for (ph, pb0, qb0, delta, taps) in groups:
            for (p, q) in taps:
                blocks.append(np.ascontiguousarray(w6s[sl, :, p, q].T))
        w8[h] = np.concatenate(blocks, axis=1).astype(fp8_np)  # [C, 36*CH]
        # pre-scaled so the epilogue computes (psum + bias*s) * (1/s)
        biass[h] = np.ascontiguousarray(
            conv_b[sl].astype(np.float32) / _CACHE["inv_scale"]
        ).reshape(CH, 1)

    r0 = loc[:, 1].astype(np.int64) * W // 1024 - L   # H-dim start (from loc x)
    c0 = loc[:, 0].astype(np.int64) * W // 1024 - L   # W-dim start (from loc y)

    in_maps = []
    for c in range(NCORES):
        j, h = c % B, c // B
        in_maps.append({
            "tissue": tissue_features[j, CH * h:CH * (h + 1)],
            "cell": cell_poly,
            "w8": w8[h],
            "bias": biass[h],
            "roff": np.array([[r0[j], c0[j]]], np.int32),
        })
    return in_maps


def run_device(tissue_features, cell_features, loc, conv_w, conv_b, **spmd_kwargs):
    """Build+run the SPMD kernel; returns (final (4,128,256,256), raw results)."""
    *_, run_bass_kernel_spmd = _get_modules()
    in_maps = _prep_inputs(tissue_features, cell_features, loc, conv_w, conv_b)
    # inv_scale is baked into the DVE epilogue as an immediate: rebuild if
    # a new weight tensor lands on a different power-of-two scale
    if _CACHE.get("built_scale") not in (None, _CACHE["inv_scale"]):
        _CACHE.pop("nc", None)
    nc = _build_program()
    _CACHE["built_scale"] = _CACHE["inv_scale"]
    res = run_bass_kernel_spmd(nc, in_maps, list(range(NCORES)), **spmd_kwargs)
    final = np.empty((B, C, H, W), np.float32)
    for c in range(NCORES):
        j, h = c % B, c // B
        final[j, CH * h:CH * (h + 1)] = res.results[c]["out"]
    return final, res


def kernel(tissue_features, cell_features, loc, conv_w, conv_b):
    final, _ = run_device(tissue_features, cell_features, loc, conv_w, conv_b)
    # reference stacks B copies of the fully-mutated tissue
    return np.broadcast_to(final[None], (B, B, C, H, W))


# revision 8
# speedup vs baseline: 1.4796x; 1.0018x over previous
"""Trainium2 Bass kernel for nn_Cell2Tissue (scatter_memory).

Reference computation:
  avg = AvgPool4x4(Conv3x3_SAME(cell) + bias)          # (128, 64, 64)
  for each tissue sample j: ROI_j += avg               # 64x64 ROI from loc
  output = stack of B copies of the mutated tissue     # (4, 4, 128, 256, 256)

Sharding over 8 cores: core c = (sample j = c % 4, channel half h = c // 4).
Each core streams its 16MB tissue half to the output and adds its half of
avg into the dynamic 64x64 ROI. The x4 output stack is a zero-copy host
broadcast at unshard time.

Key optimizations vs the naive staging (213us -> ~126us measured):
  - bulk tissue->out copy is DRAM->DRAM: each byte occupies a DMA engine
    once instead of twice (via-SBUF 2-leg copy measured ~150 GB/s payload,
    single-queue D2D 264 GB/s, concurrent dual-queue D2D ~326 GB/s).
  - cell planes travel as fp8 (half the bytes) and are enqueued FIRST on
    both HWDGE queues; the conv chases the plane arrivals and finishes
    inside the copy window instead of serializing a ~55us tail.
  - conv runs fp8 DoubleRow matmuls: taps processed in pairs via a
    custom-stride ktile dim on the moving operand (2 taps per PE pass,
    144 passes total; the 4 corner-plane single taps pair ACROSS planes
    via a plane-pitch stride). Weights pre-scaled by a power of two into
    fp8 range; the DVE epilogue rescales while adding bias. End-to-end
    rel err ~1.6e-3 (gate 2e-2).
  - the ROI scatter is reformulated as a full-width 64-row *band*:
    tissue rows [r, r+64) are loaded early (64KB contiguous per channel),
    avg lands into the band at a dynamic column offset on the DVE
    (register-offset APs), and the band is written back whole after the
    copy. This replaces 8192 x 256B scattered ROI packets (~480
    engine-us of packet-rate-bound DMA) with ~8MB of large-packet
    traffic.
  - conv is BANK-MAJOR over 8 PSUM banks so each bank's DVE epilogue
    pipelines under the remaining matmuls.

Hardware behaviors this layout works around (measured on the axon trn2):
  - per-packet round-robin between queues lets 64KB packets starve 4KB
    plane packets ~15:1; plane loads therefore go first on both queues.
  - DGE completion semaphores tick per packet, so tile-level DMA->DMA
    gates open early; only the all-engine barrier orders reliably.
  - dynamic (register-offset) DMA streams run ~200-400 GB/s and
    serialize against each other; the band write stays one-per-queue.
"""

import os
import numpy as np

B, C, H, W = 4, 128, 256, 256
CH = C // 2          # channels per core (half)
L = 32               # half ROI width
ROI = 2 * L          # 64
NCORES = 8
PRR = 65             # polyphase plane rows (max y+pb = 64)
PRC = 66             # polyphase plane cols
PHASES = 16

_CACHE = {}


def _get_modules():
    if "mods" in _CACHE:
        return _CACHE["mods"]
    if os.environ.get("JAX_PLATFORMS") in ("cpu",):
        del os.environ["JAX_PLATFORMS"]
    import concourse.bass as bass
    import concourse.mybir as mybir
    import concourse.tile as tile
    from concourse.bass_utils import run_bass_kernel_spmd

    _CACHE["mods"] = (bass, mybir, tile, run_bass_kernel_spmd)
    return _CACHE["mods"]


def _split_multiwaits(nc, mybir, max_waits=1):
    """The walrus build here rejects >1 sem-wait on some instructions (the
    Tile tail InstDrain). Hoist extra waits onto single-wait nops placed
    immediately before, on the same engine (same-engine program order
    preserves semantics)."""
    for fn in nc.m.functions:
        for bb in fn.blocks:
            insts = bb.instructions
            i = 0
            while i < len(insts):
                inst = insts[i]
                si = inst.sync_info
                if si is not None and si.on_wait and len(si.on_wait) > max_waits:
                    waits = list(si.on_wait)
                    keep = waits[-max_waits:]
                    for k, w in enumerate(waits[:-max_waits]):
                        nop = mybir.InstNoOp(
                            name=f"{inst.name}_hoistwait_{k}",
                            sync_info=mybir.SyncInfo(on_wait=[w], on_update=[]),
                            bass_nofuse=True,
                            engine=inst.engine,
                        )
                        insts.insert(i, nop)
                        i += 1
                    si.on_wait = keep
                i += 1


# planes 0-7 stream on sync, 8-15 on scalar; consume in arrival order
_PLANE_ORDER = [0, 8, 1, 9, 2, 10, 3, 11, 4, 12, 5, 13, 6, 14, 7, 15]
_PLANE_PITCH = PRR * PRC


def _conv_groups():
    """Matmul group list in plane-arrival order. Each group:
    (ph, pb0, qb0, delta_elems, taps); taps is 1 or 2 (p, q) filter taps;
    for pairs, tap[i]'s moving operand sits at base + i*delta. The four
    corner planes (one tap each) are paired ACROSS adjacent planes via a
    plane-pitch ktile stride, so every PE pass is a DoubleRow pair."""
    rank = {ph: i for i, ph in enumerate(_PLANE_ORDER)}
    groups = []  # (ready_rank, ph, pb0, qb0, delta, taps)
    singles = []
    for pp in range(4):
        for qq in range(4):
            ph = pp * 4 + qq
            ps = [p for p in (pp, pp + 4) if p < 6]
            qs = [q for q in (qq, qq + 4) if q < 6]
            taps = [(p, q) for p in ps for q in qs]
            if len(taps) == 4:
                for q in qs:
                    groups.append((rank[ph], ph, 0, q // 4, PRC,
                                   [(pp, q), (pp + 4, q)]))
            elif len(taps) == 2:
                if len(ps) == 2:   # row pair
                    groups.append((rank[ph], ph, 0, 0, PRC,
                                   [(ps[0], qs[0]), (ps[1], qs[0])]))
                else:              # column pair
                    groups.append((rank[ph], ph, 0, 0, 1,
                                   [(ps[0], qs[0]), (ps[0], qs[1])]))
            else:
                singles.append((ph, taps[0]))
    # corner singles: (10,(2,2))+(11,(2,3)) and (14,(3,2))+(15,(3,3));
    # adjacent planes, one plane-pitch apart, same (pb,qb)=(0,0)
    for (ph_a, tap_a), (ph_b, tap_b) in zip(singles[0::2], singles[1::2]):
        assert ph_b == ph_a + 1
        groups.append((max(rank[ph_a], rank[ph_b]), ph_a, 0, 0,
                       _PLANE_PITCH, [tap_a, tap_b]))
    groups.sort(key=lambda g: g[0])
    groups = [g[1:] for g in groups]
    assert sum(len(g[4]) for g in groups) == 36
    assert all(len(g[4]) == 2 for g in groups)
    return groups


def _build_program():
    """One SPMD program: per-core inputs
      tissue (64,256,256) f32, cell (128,16,65,66) fp8 polyphase
      (replicated), w8 (128, 36*64) fp8 in _conv_groups tap order,
      biass (64,1) f32 = bias * weight_scale, roff (1,2) i32 = [r0, c0]
    output: out (64,256,256) f32 = tissue with avg added in the ROI."""
    if "nc" in _CACHE:
        return _CACHE["nc"]
    bass, mybir, tile, _ = _get_modules()
    f32 = mybir.dt.float32
    fp8 = mybir.dt.float8e4
    i32 = mybir.dt.int32

    nc = bass.Bass("TRN2", target_bir_lowering=False, debug=False,
                   num_devices=NCORES)
    tissue_d = nc.dram_tensor("tissue", (CH, H, W), f32, kind="ExternalInput").ap()
    cell_d = nc.dram_tensor("cell", (C, PHASES, PRR, PRC), fp8,
                            kind="ExternalInput").ap()
    w8_d = nc.dram_tensor("w8", (C, 36 * CH), fp8, kind="ExternalInput").ap()
    bias_d = nc.dram_tensor("bias", (CH, 1), f32, kind="ExternalInput").ap()
    roff_d = nc.dram_tensor("roff", (1, 2), i32, kind="ExternalInput").ap()
    out_d = nc.dram_tensor("out", (CH, H, W), f32, kind="ExternalOutput").ap()

    NBLK = 8             # conv row blocks: 8 output rows, one PSUM bank each
    ORB = ROI // NBLK    # 8 output rows per block
    groups = _conv_groups()
    inv_s = float(_CACHE["inv_scale"])

    with tile.TileContext(nc) as tc:
        with (
            tc.tile_pool(name="const", bufs=1) as constp,
            tc.tile_pool(name="cellp", bufs=1) as cellp,
            tc.tile_pool(name="bandp", bufs=1) as bandp,
            tc.tile_pool(name="tmp", bufs=2) as tmpp,
            tc.tile_pool(name="psum", bufs=1, space="PSUM") as psump,
        ):
            # --- small consts first on the scalar (Act HWDGE) queue ---
            roff_sb = constp.tile([1, 2], i32)
            nc.scalar.dma_start(roff_sb[:], roff_d[:])
            w_sb = constp.tile([C, 36 * CH], fp8)
            nc.scalar.dma_start(w_sb[:], w8_d[:])
            bias_sb = constp.tile([CH, 1], f32)
            nc.scalar.dma_start(bias_sb[:], bias_d[:])

            # --- cell planes: first bulk traffic on both HWDGE queues ---
            cell_t = cellp.tile([C, PHASES * PRR * PRC], fp8)
            c4 = cell_t.rearrange("c (ph r w) -> c ph r w", r=PRR, w=PRC)
            for ph in range(8):
                nc.sync.dma_start(c4[:, ph], cell_d[:, ph])
            for ph in range(8, PHASES):
                nc.scalar.dma_start(c4[:, ph], cell_d[:, ph])

            # offsets are in-bounds by construction; the runtime assert's
            # ISA op miscompiles on this walrus build
            dyn_engines = (mybir.EngineType.SP, mybir.EngineType.Activation,
                           mybir.EngineType.DVE)
            r_v = nc.values_load(roff_sb[0:1, 0:1], engines=dyn_engines,
                                 min_val=0, max_val=H - ROI,
                                 skip_runtime_bounds_check=True)
            c_v = nc.values_load(roff_sb[0:1, 1:2], engines=dyn_engines,
                                 min_val=0, max_val=W - ROI,
                                 skip_runtime_bounds_check=True)

            # --- every big-packet stream is gated on the LAST plane: 64KB
            # packets starve the 4KB plane packets ~15:1 in per-packet RR,
            # so nothing big may enter a queue until the planes land.
            # A 1-byte read of plane 15's tile makes Tile emit the wait. ---
            sy_gate = constp.tile([1, 1], fp8)
            nc.sync.dma_start(sy_gate[:], c4[0:1, 15, 0:1, 0:1])

            # --- band: tissue rows [r, r+64) full width, 64KB contiguous
            # per channel ---
            band = bandp.tile([CH, ROI * W], f32)
            b3 = band.rearrange("c (r w) -> c r w", w=W)
            nc.sync.dma_start(band[:], tissue_d[:, bass.ds(r_v, ROI), :])

            # --- bulk copy tissue -> out: DRAM->DRAM, split across both
            # HWDGE queues (concurrent dual-queue D2D measured ~326 GB/s;
            # gpsimd sw-DGE ignores the gate and starves the planes, so it
            # gets none) ---
            t2 = tissue_d.rearrange("c h w -> (c h) w")
            o2 = out_d.rearrange("c h w -> (c h) w")
            R_SY = 6144                        # sync 6MB (+4MB band), scalar 10MB
            nc.sync.dma_start(o2[0:R_SY].flatten(), t2[0:R_SY].flatten())
            nc.scalar.dma_start(o2[R_SY:].flatten(), t2[R_SY:].flatten())



            # --- conv: fp8 DoubleRow tap pairs, BANK-MAJOR so each PSUM
            # bank completes ~7.5us after the previous one and its DVE
            # epilogue pipelines underneath the remaining matmuls ---
            pss = [psump.tile([CH, ORB * ROI], f32, name=f"bank{b}")
                   for b in range(NBLK)]
            avg_f = tmpp.tile([CH, ROI * ROI], f32)
            ng = len(groups)
            for b in range(NBLK):
                woff = 0
                for gi, (ph, pb0, qb0, delta, taps) in enumerate(groups):
                    lhsT = w_sb[:, woff:woff + 2 * CH].rearrange(
                        "c (k m) -> c k m", k=2)
                    rhs = c4[:, ph, b * ORB + pb0:b * ORB + pb0 + ORB,
                             qb0:qb0 + ROI].unsqueeze(1)
                    rhs.ap[1] = [delta, 2]
                    nc.tensor.matmul(
                        pss[b][:], lhsT, rhs,
                        start=(gi == 0), stop=(gi == ng - 1),
                        perf_mode=mybir.MatmulPerfMode.DoubleRow,
                    )
                    woff += 2 * CH
                # epilogue part 1 (static, pipelined under the matmuls):
                # avg_f[b] = psum[b]/s + bias. Dynamic-AP DVE ops carry
                # ~2.4us setup each, so the dynamic landing happens ONCE
                # after the loop instead of once per bank.
                nc.vector.tensor_scalar(
                    avg_f[:, b * ORB * ROI:(b + 1) * ORB * ROI],
                    pss[b][:], bias_sb[:], inv_s,
                    mybir.AluOpType.add, mybir.AluOpType.mult,
                )

            # epilogue part 2: one dynamic-offset landing of the whole
            # 64x64 result into the band
            a3 = avg_f.rearrange("c (r w) -> c r w", w=ROI)
            dstb = b3[:, :, bass.ds(c_v, ROI)]
            nc.vector.tensor_add(dstb, dstb, a3[:])

            # --- write the band after all bulk copy writes landed ---
            tc.strict_bb_all_engine_barrier()
            band_dst = out_d[:, bass.ds(r_v, ROI), :]
            for eng, ca, cb in [(nc.sync, 0, 32), (nc.scalar, 32, 64)]:
                eng.dma_start(band_dst[ca:cb], b3[ca:cb],
                              single_packet=True)

    _split_multiwaits(nc, mybir)
    # gpsimd issues no DMA in this kernel; its InstDrain (an expensive
    # dge_drain) is vacuous — swap for NoOps, keeping the sem updates
    for fn in nc.m.functions:
        for bb in fn.blocks:
            insts = bb.instructions
            for i, inst in enumerate(insts):
                if (isinstance(inst, mybir.InstDrain)
                        and inst.engine == mybir.EngineType.Pool):
                    insts[i] = mybir.InstNoOp(
                        name=f"{inst.name}_nodrain",
                        sync_info=inst.sync_info,
                        bass_nofuse=True,
                        engine=inst.engine,
                    )
    _CACHE["nc"] = nc
    return nc


def _prep_inputs(tissue_features, cell_features, loc, conv_w, conv_b):
    import ml_dtypes

    fp8_np = ml_dtypes.float8_e4m3fn
    # fold AvgPool4x4 into the conv kernel: 6x6 taps
    w6 = np.zeros((C, C, 6, 6), np.float32)
    for dr in range(4):
        for dc in range(4):
            w6[:, :, dr:dr + 3, dc:dc + 3] += conv_w
    w6 *= 1.0 / 16.0

    # scale weights into fp8 range by a power of two
    wmax = float(np.abs(w6).max())
    scale = 2.0 ** int(np.floor(np.log2(224.0 / max(wmax, 1e-30))))
    _CACHE["inv_scale"] = 1.0 / scale
    w6s = w6 * scale

    # polyphase split of the zero-padded cell map:
    # plane (pp,qq)[y,x] = padded[4y+pp, 4x+qq], padded = 1px zero border
    padc = np.zeros((C, 4 * PRC, 4 * PRC), np.float32)
    padc[:, 1:1 + H, 1:1 + W] = cell_features[0]
    cell_poly = np.empty((C, PHASES, PRR, PRC), np.float32)
    for pp in range(4):
        for qq in range(4):
            cell_poly[:, pp * 4 + qq] = padc[:, pp:pp + 4 * PRR:4, qq::4]
    cell_poly = np.ascontiguousarray(cell_poly).astype(fp8_np)

    groups = _conv_groups()
    w8 = {}
    biass = {}
    for h in range(2):
        sl = slice(CH * h, CH * (h + 1))
        blocks = []
        for (ph, pb0, qb0, delta, taps) in groups:
            for (p, q) in taps:
                blocks.append(np.ascontiguousarray(w6s[sl, :, p, q].T))
        w8[h] = np.concatenate(blocks, axis=1).astype(fp8_np)  # [C, 36*CH]
        # pre-scaled so the epilogue computes (psum + bias*s) * (1/s)
        biass[h] = np.ascontiguousarray(
            conv_b[sl].astype(np.float32) / _CACHE["inv_scale"]
        ).reshape(CH, 1)

    r0 = loc[:, 1].astype(np.int64) * W // 1024 - L   # H-dim start (from loc x)
    c0 = loc[:, 0].astype(np.int64) * W // 1024 - L   # W-dim start (from loc y)

    in_maps = []
    for c in range(NCORES):
        j, h = c % B, c // B
        in_maps.append({
            "tissue": tissue_features[j, CH * h:CH * (h + 1)],
            "cell": cell_poly,
            "w8": w8[h],
            "bias": biass[h],
            "roff": np.array([[r0[j], c0[j]]], np.int32),
        })
    return in_maps


def run_device(tissue_features, cell_features, loc, conv_w, conv_b, **spmd_kwargs):
    """Build+run the SPMD kernel; returns (final (4,128,256,256), raw results)."""
    *_, run_bass_kernel_spmd = _get_modules()
    in_maps = _prep_inputs(tissue_features, cell_features, loc, conv_w, conv_b)
    # inv_scale is baked into the DVE epilogue as an immediate: rebuild if
    # a new weight tensor lands on a different power-of-two scale
    if _CACHE.get("built_scale") not in (None, _CACHE["inv_scale"]):
        _CACHE.pop("nc", None)
    nc = _build_program()
    _CACHE["built_scale"] = _CACHE["inv_scale"]
    res = run_bass_kernel_spmd(nc, in_maps, list(range(NCORES)), **spmd_kwargs)
    final = np.empty((B, C, H, W), np.float32)
    for c in range(NCORES):
        j, h = c % B, c // B
        final[j, CH * h:CH * (h + 1)] = res.results[c]["out"]
    return final, res


def kernel(tissue_features, cell_features, loc, conv_w, conv_b):
    final, _ = run_device(tissue_features, cell_features, loc, conv_w, conv_b)
    # reference stacks B copies of the fully-mutated tissue
    return np.broadcast_to(final[None], (B, B, C, H, W))
